# revision 26
# baseline (speedup 1.0000x reference)
"""Causal single-head attention on 8 Trainium2 NeuronCores (Bass/Tile).

Problem: X [4, 2048, 1024] f32; W_q/W_k/W_v [1024, 1024] f32.
out[b] = softmax(mask((X[b] Wq)(X[b] Wk)^T / 32)) (X[b] Wv)

Sharding: 8 cores = 4 batches x 2 key-parity halves (partial softmax).
Core c = 2b + h owns batch b's key tiles {2j + h : j = 0..7} (128-row
tiles, interleaved so causal work per local tile j is j-independent across
cores). Each core projects K/V only for its own key tiles (K/V computed
once globally; only Q is duplicated - the cheapest possible duplication),
computes unnormalized partial attention over its keys, and returns the
partial numerator [2048, 1024] plus partial softmax denominators. The host
adds each pair's partials and divides. Since exp needs no max-subtraction
here (|scores/32| < ~4), partial softmax combines exactly.

One uniform SPMD program: per-core differences live in data only (which
key columns of X^T arrive in `xk`, and a [128, 256] 0/1 band mask whose
content encodes the core's parity for the diagonal score tiles).

All matmul contractions keep the contracted dim on partitions:
  A: KT[e,k] = Wk^T Xk^T, QT[e,q] = Wq^T X^T (weights as lhsT),
     V[k,e] = Xk Wv (Xk^T chunks as lhsT). bf16 in/out, fp32 PSUM.
  B: sT[k,q] = KT-tile^T @ QT (scores transposed: own keys on partitions)
     w = exp(sT/32) * band   (band only on the two diagonal q-tiles)
     den[q] = ones-matmul over w; num[q,e] = w-as-lhsT @ V
The transposed-score layout makes the attention weights directly usable as
matmul lhsT for the numerator - no on-chip transposes at all.
"""

import sys

if "/opt/trn_rl_repo" not in sys.path:
    sys.path.insert(0, "/opt/trn_rl_repo")

import numpy as np

B, S, D = 4, 2048, 1024
HK = S // 2  # own key rows per core
P = 128
N_CORES = 8
# column offset of attention-weight block j inside the packed wt tile
WOFF = [0] * 9
for _j in range(8):
    WOFF[_j + 1] = WOFF[_j] + (16 - 2 * _j) * P
WTW = WOFF[8]  # 9216

_cache = {}


def _build_nc():
    from concourse import bacc
    import concourse.mybir as mybir
    import concourse.tile as tile

    fp32 = mybir.dt.float32
    bf16 = mybir.dt.bfloat16
    Exp = mybir.ActivationFunctionType.Exp

    nc = bacc.Bacc("TRN2", target_bir_lowering=False)

    xk_d = nc.dram_tensor("xk", [D, HK], bf16, kind="ExternalInput")
    xq_d = nc.dram_tensor("xq", [D, S], bf16, kind="ExternalInput")
    wq_d = nc.dram_tensor("wq", [D, D], bf16, kind="ExternalInput")
    wk_d = nc.dram_tensor("wk", [D, D], bf16, kind="ExternalInput")
    wv_d = nc.dram_tensor("wv", [D, D], bf16, kind="ExternalInput")
    band_d = nc.dram_tensor("band", [P, 256], bf16, kind="ExternalInput")
    # num columns 0:1024 = partial numerator; column 1024 = denominator
    num_d = nc.dram_tensor("num", [S, D + 1], fp32, kind="ExternalOutput")

    xk3 = xk_d.rearrange("(o p) s -> p o s", p=P)
    xq3 = xq_d.rearrange("(o p) q -> p o q", p=P)
    wq3 = wq_d.rearrange("(o p) e -> p o e", p=P)
    wk3 = wk_d.rearrange("(o p) e -> p o e", p=P)
    wv3 = wv_d.rearrange("(o p) e -> p o e", p=P)

    with tile.TileContext(nc) as tc:
        with (
            tc.tile_pool(name="persist", bufs=1) as persist,
            tc.tile_pool(name="psA", bufs=2, space="PSUM") as psA,
            tc.tile_pool(name="psS", bufs=2, space="PSUM") as psS,
            tc.tile_pool(name="psAV", bufs=3, space="PSUM") as psAV,
            tc.tile_pool(name="psD", bufs=1, space="PSUM") as psD,
        ):
            QT = persist.tile([P, 8, S], bf16, tag="qt")
            KT = persist.tile([P, 8, HK], bf16, tag="kt")
            V = persist.tile([P, 8, D], bf16, tag="v")
            band = persist.tile([P, 256], bf16, tag="band")
            ones = persist.tile([P, 1], bf16, tag="ones")
            nc.vector.memset(ones[:], 1.0)

            # ---- Phase A: projections (all bf16 matmuls, fp32 PSUM) ----
            with (
                tc.tile_pool(name="wts", bufs=1) as wp,
                tc.tile_pool(name="xts", bufs=2) as xtsp,
                tc.tile_pool(name="warm", bufs=1) as warmp,
            ):
                Wk = wp.tile([P, 8, D], bf16, tag="wk")
                Wv = wp.tile([P, 8, D], bf16, tag="wv")
                Wq = wp.tile([P, 8, D], bf16, tag="wq")

                # PE warm-up: the cost of a matmul is halved only after ~3us
                # of continuous PE activity. The input DMAs take ~6us, so run
                # throwaway matmuls on scratch tiles meanwhile - the real
                # matmuls then start already at full clock.
                wl = warmp.tile([P, P], bf16, tag="warm_l")
                wr = warmp.tile([P, 512], bf16, tag="warm_r")
                nc.vector.memset(wl[:], 0.0)
                nc.vector.memset(wr[:], 0.0)
                ps_w = psAV.tile([P, 512], fp32, tag="psAV", name="warm")
                for _ in range(13):
                    nc.tensor.matmul(ps_w[:], wl[:], wr[:], start=True, stop=True)

                for sc in range(2):
                    xs = xtsp.tile([P, 8, 512], bf16, tag="xts")
                    if sc == 0:
                        nc.sync.dma_start(xs[:, :4], xk3[:, :4, :512])
                        nc.sync.dma_start(xs[:, 4:], xk3[:, 4:, :512])
                    else:
                        nc.sync.dma_start(
                            xs[:], xk3[:, :, sc * 512 : (sc + 1) * 512]
                        )
                    # DMA-issue-order pacing: X chunk first, then the weights
                    # needed soonest; the rest trickle in behind compute.
                    # Batched DMAs: each dma_start costs ~650ns of serial
                    # queue-issue, so fewer+bigger wins.
                    if sc == 0:
                        # Wk in pieces: the first e-tile's matmuls need
                        # only the first 128 columns, so PE starts early.
                        nc.sync.dma_start(Wk[:, :, :P], wk3[:, :, :P])
                        nc.sync.dma_start(Wk[:, :, P:512], wk3[:, :, P:512])
                        nc.sync.dma_start(Wk[:, :, 512:], wk3[:, :, 512:])
                        nc.sync.dma_start(Wv[:], wv3[:])
                    elif sc == 1:
                        nc.sync.dma_start(Wq[:], wq3[:])
                        nc.sync.dma_start(band[:], band_d[:])
                    # KT[e, k-chunk] = Wk^T @ Xk^T chunk
                    for e in range(8):
                        psum = psA.tile([P, 512], fp32, tag="psA")
                        for d in range(8):
                            nc.tensor.matmul(
                                psum[:],
                                Wk[:, d, e * P : (e + 1) * P],
                                xs[:, d],
                                start=(d == 0),
                                stop=(d == 7),
                            )
                        nc.any.tensor_copy(
                            out=KT[:, e, sc * 512 : (sc + 1) * 512], in_=psum[:]
                        )
                    # V[k-tile, e] = Xk chunk @ Wv  (Xk^T slice as lhsT)
                    for kti in range(4):
                        kt = 4 * sc + kti
                        for ec in range(2):
                            psum = psA.tile([P, 512], fp32, tag="psA")
                            for d in range(8):
                                nc.tensor.matmul(
                                    psum[:],
                                    xs[:, d, kti * P : (kti + 1) * P],
                                    Wv[:, d, ec * 512 : (ec + 1) * 512],
                                    start=(d == 0),
                                    stop=(d == 7),
                                )
                            nc.any.tensor_copy(
                                out=V[:, kt, ec * 512 : (ec + 1) * 512], in_=psum[:]
                            )
                # QT[e, q-chunk] = Wq^T @ X^T chunk (all 2048 query rows)
                for qsc in range(4):
                    xs = xtsp.tile([P, 8, 512], bf16, tag="xts")
                    nc.sync.dma_start(xs[:], xq3[:, :, qsc * 512 : (qsc + 1) * 512])
                    for e in range(8):
                        psum = psA.tile([P, 512], fp32, tag="psA")
                        for d in range(8):
                            nc.tensor.matmul(
                                psum[:],
                                Wq[:, d, e * P : (e + 1) * P],
                                xs[:, d],
                                start=(d == 0),
                                stop=(d == 7),
                            )
                        nc.any.tensor_copy(
                            out=QT[:, e, qsc * 512 : (qsc + 1) * 512], in_=psum[:]
                        )

            # ---- Phase B: partial attention over own key tiles ----
            with (
                tc.tile_pool(name="wtp", bufs=1) as wtp,
                tc.tile_pool(name="outp", bufs=2) as outp,
            ):
                # Interleaved: after key tile j's scores are exp'd, emit the
                # numerator/denominator for global q-tiles g = 2j and 2j+1
                # (they need only key tiles <= j). Keeps PE dense and spreads
                # the PSUM->SBUF copies across the whole phase.
                wt = wtp.tile([P, WTW], bf16, tag="wt")
                for j in range(8):
                    # scores + exp for own key tile j; q-range [256j, 2048)
                    for ch in range(8 - j):
                        q0 = 256 * j + 256 * ch
                        psum_s = psS.tile([P, 256], fp32, tag="psS")
                        for e in range(8):
                            nc.tensor.matmul(
                                psum_s[:],
                                KT[:, e, j * P : (j + 1) * P],
                                QT[:, e, q0 : q0 + 256],
                                start=(e == 0),
                                stop=(e == 7),
                            )
                        wcol = WOFF[j] + 256 * ch
                        nc.scalar.activation(
                            wt[:, wcol : wcol + 256], psum_s[:], Exp, scale=1 / 32.0
                        )
                        if ch == 0:
                            # diagonal block: causal 0/1 mask (parity in data)
                            nc.vector.tensor_mul(
                                wt[:, wcol : wcol + 256],
                                wt[:, wcol : wcol + 256],
                                band[:],
                            )
                    for g in (2 * j, 2 * j + 1):
                        nj = g // 2 + 1  # own key tiles jj with 2jj <= g
                        out_sb = outp.tile([P, D + 1], fp32, tag="out")
                        psum_dn = psD.tile([P, 1], fp32, tag="psD")
                        for jj in range(nj):
                            nc.tensor.matmul(
                                psum_dn[:],
                                wt[:, WOFF[jj] + (g - 2 * jj) * P :][:, :P],
                                ones[:],
                                start=(jj == 0),
                                stop=(jj == nj - 1),
                            )
                        nc.any.tensor_copy(out=out_sb[:, D : D + 1], in_=psum_dn[:])
                        for ec in range(2):
                            psum_av = psAV.tile([P, 512], fp32, tag="psAV")
                            for jj in range(nj):
                                nc.tensor.matmul(
                                    psum_av[:],
                                    wt[:, WOFF[jj] + (g - 2 * jj) * P :][:, :P],
                                    V[:, jj, ec * 512 : (ec + 1) * 512],
                                    start=(jj == 0),
                                    stop=(jj == nj - 1),
                                )
                            nc.any.tensor_copy(
                                out=out_sb[:, ec * 512 : (ec + 1) * 512],
                                in_=psum_av[:],
                            )
                            # e-half DMA right after its copy: the final
                            # copy->DMA chains overlap instead of serializing
                            if ec == 0:
                                nc.sync.dma_start(
                                    num_d[g * P : (g + 1) * P, :512],
                                    out_sb[:, :512],
                                )
                        nc.sync.dma_start(
                            num_d[g * P : (g + 1) * P, 512:], out_sb[:, 512:]
                        )

    nc.compile()
    return nc


def _get_nc():
    if "nc" not in _cache:
        _cache["nc"] = _build_nc()
    return _cache["nc"]


def _parity_cols(h):
    return np.concatenate(
        [np.arange(P * (2 * j + h), P * (2 * j + h) + P) for j in range(8)]
    )


def kernel(X, W_q, W_k, W_v, _run_kwargs=None, _results_out=None):
    import ml_dtypes
    from concourse.bass_utils import run_bass_kernel_spmd

    bf = ml_dtypes.bfloat16
    X = np.asarray(X, dtype=np.float32)
    wq16 = np.asarray(W_q, dtype=np.float32).astype(bf)
    wk16 = np.asarray(W_k, dtype=np.float32).astype(bf)
    wv16 = np.asarray(W_v, dtype=np.float32).astype(bf)

    xqs = [np.ascontiguousarray(X[b].T).astype(bf) for b in range(B)]
    cols = [_parity_cols(0), _parity_cols(1)]
    bands = []
    for h in range(2):
        x = np.arange(256)[None, :]
        p = np.arange(P)[:, None]
        bands.append((x >= p + P * h).astype(bf))

    in_maps = []
    for c in range(N_CORES):
        b, h = divmod(c, 2)
        in_maps.append(
            {
                "xk": np.ascontiguousarray(xqs[b][:, cols[h]]),
                "xq": xqs[b],
                "wq": wq16,
                "wk": wk16,
                "wv": wv16,
                "band": bands[h],
            }
        )

    nc = _get_nc()
    res = None
    for attempt in range(3):
        try:
            res = run_bass_kernel_spmd(
                nc, in_maps, core_ids=list(range(N_CORES)), **(_run_kwargs or {})
            )
            # materialize now: device failures surface lazily at fetch time,
            # and they must land inside this retry loop
            res.results = [
                {k: np.asarray(v) for k, v in r.items()} for r in res.results
            ]
            break
        except Exception:
            # transient device wedges (NRT_EXEC_UNIT_UNRECOVERABLE) usually
            # clear on retry; drop the poisoned PJRT client first
            if attempt == 2:
                raise
            print(f"kernel: device run failed (attempt {attempt}), retrying",
                  file=sys.stderr)
            import time

            try:
                import jax
                import jax.extend.backend

                jax.clear_caches()
                jax.extend.backend.clear_backends()
            except Exception:
                pass
            time.sleep(3)
    if _results_out is not None:
        _results_out.append(res)

    out = np.empty((B, S, D), dtype=np.float32)
    for b in range(B):
        buf = res.results[2 * b]["num"] + res.results[2 * b + 1]["num"]
        out[b] = buf[:, :D] / buf[:, D:]
    return out
